# revision 32
# baseline (speedup 1.0000x reference)
"""Trainium2 Bass kernel for nn_DecoderWithAttention (8-core SPMD).

Strategy:
 - Tensor-parallel over the 4096 gate dim for the 3 LSTMs (each core owns 128
   h-dims = 512 gate cols); vocab heads column-split 1250/core.
 - Loop-invariant hoisting: img_mean/attr_mean input blocks precomputed as
   per-batch gate constants; teacher-forced embedding contributions batched in
   phase 0; v_att folded through the l2 weights (Zv trick) so v_att is never
   materialized; s_att contribution = gate * (attr_mean @ W.T) precomputed.
 - Recurrent state flows TRANSPOSED (h.T bf16 tiles [128,64]) so it is directly
   usable as matmul stationaries; two AllGathers per step exchange h slices.
 - All matmuls bf16 operands with f32 PSUM accumulation.
 - preds_v interleaved per-step (fills PE during collectives); preds_s/preds
   batched in phase 2 from h1sT/h2T history.
"""

import numpy as np
import ml_dtypes

import concourse.bass as bass
import concourse.mybir as mybir
import concourse.tile as tile
from concourse.bass_utils import run_bass_kernel_spmd
from concourse.masks import make_identity
from bass_rust import SyncInfo

BF16 = mybir.dt.bfloat16
F32 = mybir.dt.float32
AX = mybir.AluOpType
AF = mybir.ActivationFunctionType

NC = 8          # cores
B = 64          # batch
R = 36          # image regions
F = 2048        # image feature dim
D = 1024        # hidden
ECAP = 1024
EATTR = 512
AV = 512
AS = 512
VCAP = 10000
T = 19          # decode steps (LCAP-1)
GS = 512        # gate cols per core (4*128)
HS = 128        # h dims per core
VS = VCAP // NC  # 1250 vocab cols per core
NROW = (B * R + 127) // 128          # 18 row tiles of (b,r)
NROW_PAD = 24                        # padded to 3 per core
RPC = NROW_PAD // NC                 # 3 att1 row tiles per core
NPAIR = (T + 1) // 2                 # 10 step-pairs for phase 2

_NSPLIT = [(0, 512), (512, 1024), (1024, 1250)]


def _split_multi_waits(nc):
    """walrus here encodes at most ONE sync wait per instruction; hoist
    extra waits onto single-wait nops inserted just before."""
    eng_map = nc.engines
    for bb in nc.main_func.blocks:
        insts = bb.instructions
        todo = [i for i in insts if i.sync_info is not None
                and i.sync_info.on_wait and len(i.sync_info.on_wait) > 1]
        if not todo:
            continue
        nop_for = {}
        for ins in todo:
            waits = list(ins.sync_info.on_wait)
            nops = []
            for w in waits[:-1]:
                nb = eng_map[ins.engine].nop()
                nb.ins.sync_info = SyncInfo(on_wait=[w], on_update=[])
                nops.append(nb.ins)
            ins.sync_info = SyncInfo(
                on_wait=[waits[-1]],
                on_update=list(ins.sync_info.on_update or []),
            )
            nop_for[id(ins)] = nops
        created = {id(n) for ns in nop_for.values() for n in ns}
        for bb2 in nc.main_func.blocks:
            bb2.instructions[:] = [i for i in bb2.instructions
                                   if id(i) not in created]
        new = []
        for ins in insts:
            new.extend(nop_for.get(id(ins), ()))
            new.append(ins)
        insts[:] = new


def _chunked(a, p=128):
    """[K, N] -> [p, (K//p)*N] SBUF chunk-major layout."""
    K, N = a.shape
    assert K % p == 0
    return np.ascontiguousarray(
        a.reshape(K // p, p, N).transpose(1, 0, 2).reshape(p, (K // p) * N))


def _bf(a):
    return np.ascontiguousarray(a).astype(ml_dtypes.bfloat16)


def _f32(a):
    return np.ascontiguousarray(a).astype(np.float32)


# ----------------------------------------------------------------- program

def build_program():
    nc = bass.Bass(trn_type="TRN2")

    def din(name, shape, dt=BF16):
        return nc.dram_tensor(name, shape, dt, kind="ExternalInput")

    # --- inputs (per-core data, identical shapes on every core) ---
    w1v = din("w1v", [128, 16 * GS])
    w1s = din("w1s", [128, 16 * GS])
    w2 = din("w2", [128, 24 * GS])
    whv = din("whv", [128, 8 * GS])
    whs = din("whs", [128, 8 * GS])
    wvv = din("wvv", [128, 8 * VS])
    wvs_d = din("wvs_d", [128, 8 * VS])
    wvp_d = din("wvp_d", [128, 8 * VS])
    imgt = din("imgt", [F, B * R])
    imgt_att = din("imgt_att", [128, 16 * RPC * 128])
    embt = din("embt", [128, 8 * T * B])
    wembv = din("wembv", [128, 8 * GS])
    wembs = din("wembs", [128, 8 * GS])
    wimg = din("wimg", [128, 16 * GS])
    wzv = din("wzv", [128, 16 * GS])
    wf = din("wf", [128, 16 * GS])
    amt = din("amt", [128, 4 * B])
    wattr1s = din("wattr1s", [128, 4 * GS])
    wattr2 = din("wattr2", [128, 4 * GS])
    wsattx = din("wsattx", [128, 4 * GS])
    b1v = din("b1v", [B, GS], F32)
    b1s = din("b1s", [B, GS], F32)
    battv = din("battv", [128, GS], F32)
    bsatt = din("bsatt", [B, GS], F32)
    b2t = din("b2t", [128, 256], F32)
    wvtile = din("wvtile", [128, 256])
    wstile = din("wstile", [B, GS], F32)
    sbt = din("sbt", [B, 1], F32)
    maskp = din("maskp", [128, NPAIR], F32)
    foldm = din("foldm", [128, B])

    pv_out = nc.dram_tensor("pv_out", [T * B, VS], F32, kind="ExternalOutput")
    ps_out = nc.dram_tensor("ps_out", [T * B, VS], F32, kind="ExternalOutput")
    pp_out = nc.dram_tensor("pp_out", [T * B, VS], F32, kind="ExternalOutput")

    rg = [list(range(NC))]

    with tile.TileContext(nc) as tc:
        import contextlib
        est = contextlib.ExitStack()
        with est:
            dram = est.enter_context(tc.tile_pool(name="dram", bufs=1, space="DRAM"))
            dram2 = est.enter_context(tc.tile_pool(name="dram2", bufs=2, space="DRAM"))
            sbW = est.enter_context(tc.tile_pool(name="sbW", bufs=1))
            sbC = est.enter_context(tc.tile_pool(name="sbC", bufs=1))

            # persistent DRAM intermediates
            zv_nat = dram.tile([NROW * 128, GS], BF16)
            att1_nat = dram.tile([NROW_PAD * 128, GS], BF16)
            embp1v_d = dram.tile([T * B, GS], BF16)
            embp1s_d = dram.tile([T * B, GS], BF16)
            h1v_hist = dram.tile([T * 128, 8 * B], BF16)
            h1s_hist = dram.tile([T * 128, 8 * B], BF16)
            h2_hist = dram.tile([T * 128, 8 * B], BF16)

            # persistent SBUF: weight slabs + consts
            w1v_s = sbW.tile([128, 16 * GS], BF16)
            w1s_s = sbW.tile([128, 16 * GS], BF16)
            w2_s = sbW.tile([128, 24 * GS], BF16)
            whv_s = sbW.tile([128, 8 * GS], BF16)
            whs_s = sbW.tile([128, 8 * GS], BF16)
            for dst, src in ((w1v_s, w1v), (w1s_s, w1s), (w2_s, w2),
                             (whv_s, whv), (whs_s, whs)):
                nc.sync.dma_start(dst[:], src[:])

            const1v = sbC.tile([B, GS], F32)
            const1s = sbC.tile([B, GS], F32)
            attr2p = sbC.tile([B, GS], F32)
            xconst = sbC.tile([B, GS], F32)
            wvtile_s = sbC.tile([128, 256], BF16)
            wstile_s = sbC.tile([B, GS], F32)
            sbt_s = sbC.tile([B, 1], F32)
            maskp_s = sbC.tile([128, NPAIR], F32)
            foldm_s = sbC.tile([128, B], BF16)
            ident = sbC.tile([B, B], F32)
            nc.sync.dma_start(wvtile_s[:], wvtile[:])
            nc.sync.dma_start(wstile_s[:], wstile[:])
            nc.sync.dma_start(sbt_s[:], sbt[:])
            nc.sync.dma_start(maskp_s[:], maskp[:])
            nc.sync.dma_start(foldm_s[:], foldm[:])
            make_identity(nc, ident[:])

            # persistent state tiles
            c1v = sbC.tile([B, HS], F32)
            c1s = sbC.tile([B, HS], F32)
            c2 = sbC.tile([B, HS], F32)
            for t_ in (c1v, c1s, c2):
                nc.vector.memset(t_[:], 0.0)

            # ======================= PHASE 0 =======================
            with (
                tc.tile_pool(name="sbP0", bufs=1) as p0,
                tc.tile_pool(name="sbP0s", bufs=2) as p0s,
                tc.tile_pool(name="ps0", bufs=4, space="PSUM") as ps0,
            ):
                imgt_att_s = p0.tile([128, 16 * RPC * 128], BF16)
                wimg_s = p0.tile([128, 16 * GS], BF16)
                wzv_s = p0.tile([128, 16 * GS], BF16)
                wf_s = p0.tile([128, 16 * GS], BF16)
                amt_s = p0.tile([128, 4 * B], BF16)
                wattr1s_s = p0.tile([128, 4 * GS], BF16)
                wattr2_s = p0.tile([128, 4 * GS], BF16)
                wsattx_s = p0.tile([128, 4 * GS], BF16)
                b1v_s = p0.tile([B, GS], F32)
                b1s_s = p0.tile([B, GS], F32)
                battv_s = p0.tile([128, GS], F32)
                bsatt_s = p0.tile([B, GS], F32)
                b2t_s = p0.tile([128, 256], F32)
                for dst, src in ((imgt_att_s, imgt_att), (wimg_s, wimg),
                                 (wzv_s, wzv), (wf_s, wf), (amt_s, amt),
                                 (wattr1s_s, wattr1s), (wattr2_s, wattr2),
                                 (wsattx_s, wsattx), (b1v_s, b1v),
                                 (b1s_s, b1s), (battv_s, battv),
                                 (bsatt_s, bsatt), (b2t_s, b2t)):
                    nc.sync.dma_start(dst[:], src[:])

                imgmeanT = p0.tile([128, 16 * B], F32)
                imgmeanTb = p0.tile([128, 16 * B], BF16)

                # img mean (sum; 1/36 folded into wimg host-side) + Zv GEMM
                for c in range(16):
                    ch = p0s.tile([128, B * R], BF16, tag="imgch")
                    nc.sync.dma_start(ch[:], imgt[128 * c:128 * (c + 1), :])
                    nc.vector.tensor_reduce(
                        imgmeanT[:, B * c:B * (c + 1)],
                        ch[:].rearrange("p (b r) -> p b r", r=R),
                        axis=mybir.AxisListType.X, op=AX.add)
                nc.vector.tensor_copy(imgmeanTb[:], imgmeanT[:])

                # Zv: for each row tile m: accumulate over 16 k-chunks
                for m in range(NROW):
                    ps = ps0.tile([128, GS], F32, tag="p0")
                    for c in range(16):
                        st = p0s.tile([128, 128], BF16, tag="imgst")
                        nc.sync.dma_start(
                            st[:], imgt[128 * c:128 * (c + 1),
                                        128 * m:128 * (m + 1)])
                        nc.tensor.matmul(ps[:], st[:],
                                         wzv_s[:, GS * c:GS * (c + 1)],
                                         start=(c == 0), stop=(c == 15))
                    ob = p0s.tile([128, GS], BF16, tag="p0o")
                    nc.scalar.copy(ob[:], ps[:])
                    nc.sync.dma_start(zv_nat[128 * m:128 * (m + 1), :], ob[:])

                # att1 rows for this core (+ vatt bf+bh bias fold)
                ag_in = dram.tile([RPC * 128, GS], BF16)
                for rt in range(RPC):
                    ps = ps0.tile([128, GS], F32, tag="p0")
                    for c in range(16):
                        nc.tensor.matmul(
                            ps[:],
                            imgt_att_s[:, (c * RPC + rt) * 128:
                                       (c * RPC + rt) * 128 + 128],
                            wf_s[:, GS * c:GS * (c + 1)],
                            start=(c == 0), stop=(c == 15))
                    nc.vector.tensor_add(ps[:], ps[:], battv_s[:])
                    ob = p0s.tile([128, GS], BF16, tag="p0o")
                    nc.vector.tensor_copy(ob[:], ps[:])
                    nc.sync.dma_start(ag_in[128 * rt:128 * (rt + 1), :],
                                      ob[:])

                # const1v = img_mean @ Wimg.T + biases
                ps = ps0.tile([B, GS], F32, tag="p0b")
                for c in range(16):
                    nc.tensor.matmul(ps[:], imgmeanTb[:, B * c:B * (c + 1)],
                                     wimg_s[:, GS * c:GS * (c + 1)],
                                     start=(c == 0), stop=(c == 15))
                nc.vector.tensor_add(const1v[:], ps[:], b1v_s[:])

                # attr-derived consts
                ps = ps0.tile([B, GS], F32, tag="p0b")
                for c in range(4):
                    nc.tensor.matmul(ps[:], amt_s[:, B * c:B * (c + 1)],
                                     wattr1s_s[:, GS * c:GS * (c + 1)],
                                     start=(c == 0), stop=(c == 3))
                nc.vector.tensor_add(const1s[:], ps[:], b1s_s[:])
                ps = ps0.tile([B, GS], F32, tag="p0b")
                for c in range(4):
                    nc.tensor.matmul(ps[:], amt_s[:, B * c:B * (c + 1)],
                                     wattr2_s[:, GS * c:GS * (c + 1)],
                                     start=(c == 0), stop=(c == 3))
                nc.vector.tensor_copy(attr2p[:], ps[:])
                ps = ps0.tile([B, GS], F32, tag="p0b")
                for c in range(4):
                    nc.tensor.matmul(ps[:], amt_s[:, B * c:B * (c + 1)],
                                     wsattx_s[:, GS * c:GS * (c + 1)],
                                     start=(c == 0), stop=(c == 3))
                nc.vector.tensor_add(xconst[:], ps[:], bsatt_s[:])

                # att1 AllGather
                nc.gpsimd.collective_compute(
                    "AllGather", AX.bypass, replica_groups=rg,
                    ins=[ag_in.opt()], outs=[att1_nat.opt()])

            # embedding parts (separate scope to cap SBUF)
            with (
                tc.tile_pool(name="sbP0b", bufs=1) as p0b,
                tc.tile_pool(name="sbP0bs", bufs=2) as p0bs,
                tc.tile_pool(name="ps0b", bufs=2, space="PSUM") as ps0b,
            ):
                embt_s = p0b.tile([128, 8 * T * B], BF16)
                wembv_s = p0b.tile([128, 8 * GS], BF16)
                wembs_s = p0b.tile([128, 8 * GS], BF16)
                nc.sync.dma_start(embt_s[:], embt[:])
                nc.sync.dma_start(wembv_s[:], wembv[:])
                nc.sync.dma_start(wembs_s[:], wembs[:])
                nrb = (T * B + 127) // 128  # 10
                for w_s, dst in ((wembv_s, embp1v_d), (wembs_s, embp1s_d)):
                    for rb in range(nrb):
                        rw = min(128, T * B - 128 * rb)
                        ps = ps0b.tile([128, GS], F32, tag="pe")
                        for c in range(8):
                            nc.tensor.matmul(
                                ps[:rw, :],
                                embt_s[:, c * T * B + 128 * rb:
                                       c * T * B + 128 * rb + rw],
                                w_s[:, GS * c:GS * (c + 1)],
                                start=(c == 0), stop=(c == 7))
                        ob = p0bs.tile([128, GS], BF16, tag="peo")
                        nc.scalar.copy(ob[:rw, :], ps[:rw, :])
                        nc.sync.dma_start(
                            dst[128 * rb:128 * rb + rw, :], ob[:rw, :])

            # ======================= PHASE 1 =======================
            with (
                tc.tile_pool(name="sbP1", bufs=1) as p1,
                tc.tile_pool(name="sbP1r", bufs=2) as p1r,
                tc.tile_pool(name="sbE", bufs=1) as sbE,
                tc.tile_pool(name="psA", bufs=2, space="PSUM") as psA,
                tc.tile_pool(name="psB", bufs=1, space="PSUM") as psB,
                tc.tile_pool(name="psC", bufs=2, space="PSUM") as psC,
            ):
                zv_s = p1.tile([128, R * 256], BF16)
                att1_s = p1.tile([128, R * 256], BF16)
                # scatter (b,r)-major DRAM -> (b|nh, r, 256) SBUF layout
                for half in range(2):
                    for src, dst in ((zv_nat, zv_s), (att1_nat, att1_s)):
                        v = src[0:B * R, 256 * half:256 * (half + 1)]
                        v = v.rearrange("(b r) k -> b r k", r=R)
                        d = dst[B * half:B * (half + 1), :]
                        d = d.rearrange("p (r k) -> p r k", r=R)
                        nc.sync.dma_start(d, v)
                # fold bias2 into Zv
                nc.vector.tensor_add(
                    zv_s[:].rearrange("p (r k) -> p r k", r=R),
                    zv_s[:].rearrange("p (r k) -> p r k", r=R),
                    b2t_s[:].unsqueeze(1).to_broadcast((128, R, 256)))

                hT_shape = [128, 8 * B]
                h1vT = p1r.tile(hT_shape, BF16, tag="h1vT")
                h1sT = p1r.tile(hT_shape, BF16, tag="h1sT")
                h2T = p1r.tile(hT_shape, BF16, tag="h2T")
                for t_ in (h1vT, h1sT, h2T):
                    nc.vector.memset(t_[:], 0.0)

                def gate_block(ps_g, cst, hloc):
                    sfi = p1r.tile([B, 256], F32, tag="sfi")
                    nc.scalar.activation(sfi[:], ps_g[:, 0:256], AF.Sigmoid)
                    so = p1r.tile([B, HS], F32, tag="so")
                    nc.scalar.activation(so[:], ps_g[:, 384:512], AF.Sigmoid)
                    tg = p1r.tile([B, HS], F32, tag="tg")
                    nc.scalar.activation(tg[:], ps_g[:, 256:384], AF.Tanh)
                    nc.vector.tensor_mul(cst[:], sfi[:, 128:256], cst[:])
                    tmp = p1r.tile([B, HS], F32, tag="tmp")
                    nc.vector.tensor_mul(tmp[:], sfi[:, 0:128], tg[:])
                    nc.vector.tensor_add(cst[:], cst[:], tmp[:])
                    tch = p1r.tile([B, HS], F32, tag="tch")
                    nc.scalar.activation(tch[:], cst[:], AF.Tanh)
                    nc.vector.tensor_mul(hloc[:], so[:], tch[:])

                for t in range(T):
                    h1vT_p, h1sT_p, h2T_p = h1vT, h1sT, h2T

                    # l1v / l1s gates: h1-part first (independent of AG2)
                    ps_g1v = psA.tile([B, GS], F32, tag="g1")
                    ps_g1s = psA.tile([B, GS], F32, tag="g1")
                    for c in range(8):
                        nc.tensor.matmul(ps_g1v[:],
                                         h1vT_p[:, B * c:B * (c + 1)],
                                         w1v_s[:, GS * (8 + c):GS * (9 + c)],
                                         start=(c == 0), stop=False)
                    for c in range(8):
                        nc.tensor.matmul(ps_g1s[:],
                                         h1sT_p[:, B * c:B * (c + 1)],
                                         w1s_s[:, GS * (8 + c):GS * (9 + c)],
                                         start=(c == 0), stop=False)
                    for c in range(8):
                        nc.tensor.matmul(ps_g1v[:],
                                         h2T_p[:, B * c:B * (c + 1)],
                                         w1v_s[:, GS * c:GS * (c + 1)],
                                         start=False, stop=(c == 7))
                    for c in range(8):
                        nc.tensor.matmul(ps_g1s[:],
                                         h2T_p[:, B * c:B * (c + 1)],
                                         w1s_s[:, GS * c:GS * (c + 1)],
                                         start=False, stop=(c == 7))

                    # embedding part + const
                    ev = p1r.tile([B, GS], BF16, tag="ev")
                    nc.sync.dma_start(ev[:], embp1v_d[B * t:B * (t + 1), :])
                    es = p1r.tile([B, GS], BF16, tag="es")
                    nc.sync.dma_start(es[:], embp1s_d[B * t:B * (t + 1), :])
                    nc.vector.tensor_add(ps_g1v[:], ps_g1v[:], const1v[:])
                    nc.vector.tensor_add(ps_g1v[:], ps_g1v[:], ev[:])
                    nc.vector.tensor_add(ps_g1s[:], ps_g1s[:], const1s[:])
                    nc.vector.tensor_add(ps_g1s[:], ps_g1s[:], es[:])

                    h1v_loc = p1r.tile([B, HS], F32, tag="h1v_loc")
                    h1s_loc = p1r.tile([B, HS], F32, tag="h1s_loc")
                    gate_block(ps_g1v, c1v, h1v_loc)
                    gate_block(ps_g1s, c1s, h1s_loc)

                    # transpose h slices, pack bf16
                    pack1 = p1r.tile([128, 2 * B], BF16, tag="pack1")
                    tr = psC.tile([HS, B], F32, tag="att")
                    nc.tensor.transpose(tr[:], h1v_loc[:], ident[:])
                    nc.scalar.copy(pack1[:, 0:B], tr[:])
                    tr2 = psC.tile([HS, B], F32, tag="att")
                    nc.tensor.transpose(tr2[:], h1s_loc[:], ident[:])
                    nc.scalar.copy(pack1[:, B:2 * B], tr2[:])

                    # AllGather #1 (h1v.T, h1s.T slices)
                    ag1i = dram2.tile([128, 2 * B], BF16, tag="ag1i")
                    nc.sync.dma_start(ag1i[:], pack1[:])
                    ag1o = dram2.tile([NC * 128, 2 * B], BF16, tag="ag1o")
                    nc.gpsimd.collective_compute(
                        "AllGather", AX.bypass, replica_groups=rg,
                        ins=[ag1i.opt()], outs=[ag1o.opt()])
                    h1vT = p1r.tile(hT_shape, BF16, tag="h1vT")
                    h1sT = p1r.tile(hT_shape, BF16, tag="h1sT")
                    nc.sync.dma_start(
                        h1vT[:].rearrange("p (c x) -> p c x", x=B),
                        ag1o[:, 0:B].rearrange("(c p) x -> p c x", p=128))
                    nc.sync.dma_start(
                        h1sT[:].rearrange("p (c x) -> p c x", x=B),
                        ag1o[:, B:2 * B].rearrange("(c p) x -> p c x", p=128))

                    # attention GEMMs on gathered h
                    ps_att2 = psC.tile([B, GS], F32, tag="att")
                    for c in range(8):
                        nc.tensor.matmul(ps_att2[:],
                                         h1vT[:, B * c:B * (c + 1)],
                                         whv_s[:, GS * c:GS * (c + 1)],
                                         start=(c == 0), stop=(c == 7))
                    ps_a2 = psC.tile([B, GS], F32, tag="att")
                    for c in range(8):
                        nc.tensor.matmul(ps_a2[:],
                                         h1sT[:, B * c:B * (c + 1)],
                                         whs_s[:, GS * c:GS * (c + 1)],
                                         start=(c == 0), stop=(c == 7))

                    # l2 gates
                    ps_g2 = psB.tile([B, GS], F32, tag="g2")
                    for c in range(8):
                        nc.tensor.matmul(ps_g2[:], h1vT[:, B * c:B * (c + 1)],
                                         w2_s[:, GS * c:GS * (c + 1)],
                                         start=(c == 0), stop=False)
                    for c in range(8):
                        nc.tensor.matmul(ps_g2[:], h1sT[:, B * c:B * (c + 1)],
                                         w2_s[:, GS * (8 + c):GS * (9 + c)],
                                         start=False, stop=False)
                    for c in range(8):
                        nc.tensor.matmul(ps_g2[:], h2T_p[:, B * c:B * (c + 1)],
                                         w2_s[:, GS * (16 + c):GS * (17 + c)],
                                         start=False, stop=(c == 7))

                    # ---- e pipeline (visual attention scores) ----
                    att2b = p1r.tile([128, 256], BF16, tag="att2b")
                    nc.scalar.copy(att2b[0:B, :], ps_att2[:, 0:256])
                    nc.scalar.copy(att2b[B:128, :], ps_att2[:, 256:512])
                    u = sbE.tile([128, R * 256], BF16, tag="escratch")
                    nc.vector.tensor_add(
                        u[:].rearrange("p (r k) -> p r k", r=R),
                        att1_s[:].rearrange("p (r k) -> p r k", r=R),
                        att2b[:].unsqueeze(1).to_broadcast((128, R, 256)))
                    nc.vector.tensor_scalar_max(u[:], u[:], 0.0)
                    nc.vector.tensor_mul(
                        u[:].rearrange("p (r k) -> p r k", r=R),
                        u[:].rearrange("p (r k) -> p r k", r=R),
                        wvtile_s[:].unsqueeze(1).to_broadcast((128, R, 256)))
                    e2 = p1r.tile([128, R], F32, tag="e2")
                    nc.vector.tensor_reduce(
                        e2[:], u[:].rearrange("p (r k) -> p r k", r=R),
                        axis=mybir.AxisListType.X, op=AX.add)
                    e2b = p1r.tile([128, R], BF16, tag="e2b")
                    nc.vector.tensor_copy(e2b[:], e2[:])
                    # fold (b, kh-half) partitions: e[b] = e2[b] + e2[b+64]
                    e = psC.tile([B, R], F32, tag="att")
                    nc.tensor.matmul(e[:], foldm_s[:], e2b[:],
                                     start=True, stop=True)
                    # softmax over r
                    mneg = p1r.tile([B, 1], F32, tag="mneg")
                    nc.vector.tensor_reduce(mneg[:], e[:],
                                            axis=mybir.AxisListType.X,
                                            op=AX.max, negate=True)
                    ex = p1r.tile([B, R], F32, tag="ex")
                    nc.scalar.activation(ex[:], e[:], AF.Exp, bias=mneg[:])
                    ssum = p1r.tile([B, 1], F32, tag="ssum")
                    nc.vector.tensor_reduce(ssum[:], ex[:],
                                            axis=mybir.AxisListType.X,
                                            op=AX.add)
                    rcp = p1r.tile([B, 1], F32, tag="rcp")
                    nc.vector.reciprocal(rcp[:], ssum[:])
                    alpha = p1r.tile([128, R], F32, tag="alpha")
                    nc.vector.tensor_scalar_mul(alpha[0:B, :], ex[:], rcp[:])
                    nc.vector.tensor_copy(alpha[B:128, :], alpha[0:B, :])

                    # weighted Zv reduction -> v_att contribution to gates2
                    prod = sbE.tile([128, R * 256], BF16, tag="escratch")
                    nc.vector.tensor_mul(
                        prod[:].rearrange("p (r k) -> p r k", r=R),
                        zv_s[:].rearrange("p (r k) -> p r k", r=R),
                        alpha[:].unsqueeze(2).to_broadcast((128, R, 256)))
                    zr = p1r.tile([128, 256], F32, tag="zr")
                    nc.vector.tensor_reduce(
                        zr[:],
                        prod[:].rearrange("p (r k) -> p k r", r=R),
                        axis=mybir.AxisListType.X, op=AX.add)
                    nc.vector.tensor_add(ps_g2[:, 0:256], ps_g2[:, 0:256],
                                         zr[0:B, :])
                    nc.vector.tensor_add(ps_g2[:, 256:512], ps_g2[:, 256:512],
                                         zr[B:128, :])

                    # semantic gate
                    a2r = p1r.tile([B, GS], F32, tag="a2r")
                    nc.vector.tensor_add(a2r[:], ps_a2[:], xconst[:])
                    nc.scalar.activation(a2r[:], a2r[:], AF.Relu)
                    nc.vector.tensor_mul(a2r[:], a2r[:], wstile_s[:])
                    glog = p1r.tile([B, 1], F32, tag="glog")
                    nc.vector.tensor_reduce(glog[:], a2r[:],
                                            axis=mybir.AxisListType.X,
                                            op=AX.add)
                    gate = p1r.tile([B, 1], F32, tag="gate")
                    nc.scalar.activation(gate[:], glog[:], AF.Sigmoid,
                                         bias=sbt_s[:])
                    a2g = p1r.tile([B, GS], F32, tag="a2g")
                    nc.vector.tensor_scalar_mul(a2g[:], attr2p[:], gate[:])
                    nc.vector.tensor_add(ps_g2[:], ps_g2[:], a2g[:])

                    h2_loc = p1r.tile([B, HS], F32, tag="h2_loc")
                    gate_block(ps_g2, c2, h2_loc)

                    pack2 = p1r.tile([128, B], BF16, tag="pack2")
                    tr3 = psC.tile([HS, B], F32, tag="att")
                    nc.tensor.transpose(tr3[:], h2_loc[:], ident[:])
                    nc.scalar.copy(pack2[:], tr3[:])

                    ag2i = dram2.tile([128, B], BF16, tag="ag2i")
                    nc.sync.dma_start(ag2i[:], pack2[:])
                    ag2o = dram2.tile([NC * 128, B], BF16, tag="ag2o")
                    nc.gpsimd.collective_compute(
                        "AllGather", AX.bypass, replica_groups=rg,
                        ins=[ag2i.opt()], outs=[ag2o.opt()])
                    h2T = p1r.tile(hT_shape, BF16, tag="h2T")
                    nc.sync.dma_start(
                        h2T[:].rearrange("p (c x) -> p c x", x=B),
                        ag2o[:].rearrange("(c p) x -> p c x", p=128))

                    # history for phase 2
                    nc.sync.dma_start(h1v_hist[128 * t:128 * (t + 1), :],
                                      h1vT[:])
                    nc.sync.dma_start(h1s_hist[128 * t:128 * (t + 1), :],
                                      h1sT[:])
                    nc.sync.dma_start(h2_hist[128 * t:128 * (t + 1), :],
                                      h2T[:])

            # ======================= PHASE 2 =======================
            with (
                tc.tile_pool(name="sbP2", bufs=1) as p2,
                tc.tile_pool(name="sbP2s", bufs=2) as p2s,
                tc.tile_pool(name="psP", bufs=2, space="PSUM") as psP,
            ):
                wvv_s2 = p2.tile([128, 8 * VS], BF16)
                wvs_s2 = p2.tile([128, 8 * VS], BF16)
                wvp_s2 = p2.tile([128, 8 * VS], BF16)
                nc.sync.dma_start(wvv_s2[:], wvv[:])
                nc.sync.dma_start(wvs_s2[:], wvs_d[:])
                nc.sync.dma_start(wvp_s2[:], wvp_d[:])
                for hist, w_s, out_d in ((h1v_hist, wvv_s2, pv_out),
                                         (h1s_hist, wvs_s2, ps_out),
                                         (h2_hist, wvp_s2, pp_out)):
                    for p in range(NPAIR):
                        t0 = 2 * p
                        t1 = min(2 * p + 1, T - 1)
                        hp = p2s.tile([128, 8 * 128], BF16, tag="hp")
                        for c in range(8):
                            nc.sync.dma_start(
                                hp[:, 128 * c:128 * c + B],
                                hist[128 * t0:128 * (t0 + 1),
                                     B * c:B * (c + 1)])
                            nc.sync.dma_start(
                                hp[:, 128 * c + B:128 * (c + 1)],
                                hist[128 * t1:128 * (t1 + 1),
                                     B * c:B * (c + 1)])
                        ps = psP.tile([128, VS], F32, tag="pp")
                        for c in range(8):
                            for (n0, n1) in _NSPLIT:
                                nc.tensor.matmul(
                                    ps[:, n0:n1],
                                    hp[:, 128 * c:128 * (c + 1)],
                                    w_s[:, VS * c + n0:VS * c + n1],
                                    start=(c == 0), stop=(c == 7))
                        ob = p2s.tile([128, VS], F32, tag="ppo")
                        nc.vector.tensor_scalar_mul(ob[:], ps[:],
                                                    maskp_s[:, p:p + 1])
                        rw = 128 if 2 * p + 1 < T else B
                        nc.sync.dma_start(
                            out_d[B * 2 * p:B * 2 * p + rw, :], ob[:rw, :])

    _split_multi_waits(nc)
    return nc


# ----------------------------------------------------------------- host prep

def prep_inputs(image_features, encoded_captions, caption_lengths,
                encoded_attributes, params):
    p = {k: np.asarray(v) for k, v in params.items()}
    img = _f32(image_features)
    caps0 = np.asarray(encoded_captions)
    cl = np.asarray(caption_lengths)
    attrs = np.asarray(encoded_attributes)

    lengths = cl[:, 0]
    sort_ind = np.argsort(-lengths, kind="stable")
    img_s = img[sort_ind]
    caps_s = caps0[sort_ind]
    lengths_s = lengths[sort_ind]
    dec_len = lengths_s - 1

    attr_mean = _f32(p["attr_emb"])[attrs.astype(np.int64)].mean(1)  # [B,EATTR]
    emb = _f32(p["cap_emb"])[caps_s[:, :T].astype(np.int64)]  # [B,T,ECAP]
    # t-major rows (t*B + b) to match device embp layout
    emb_tb = np.ascontiguousarray(emb.transpose(1, 0, 2)).reshape(T * B, ECAP)

    mask = (np.arange(T)[None, :] < dec_len[:, None]).astype(np.float32)

    def wn(vk, gk):
        v = _f32(p[vk])
        g = _f32(p[gk])
        return v * (g / np.linalg.norm(v, axis=1))[:, None]

    wnv = wn("fcv_v", "fcv_g")
    wns = wn("fcs_v", "fcs_g")
    wnp = wn("fc_v", "fc_g")
    bv = _f32(p["fcv_b"])
    bs = _f32(p["fcs_b"])
    bp = _f32(p["fc_b"])

    l1v_Wih = _f32(p["l1v_Wih"]); l1v_Whh = _f32(p["l1v_Whh"])
    l1s_Wih = _f32(p["l1s_Wih"]); l1s_Whh = _f32(p["l1s_Whh"])
    l2_Wih = _f32(p["l2_Wih"]); l2_Whh = _f32(p["l2_Whh"])
    b1v_full = _f32(p["l1v_bih"]) + _f32(p["l1v_bhh"])
    b1s_full = _f32(p["l1s_bih"]) + _f32(p["l1s_bhh"])
    b2_full = _f32(p["l2_bih"]) + _f32(p["l2_bhh"])

    vatt_Wf = _f32(p["vatt_Wf"]); vatt_Wh = _f32(p["vatt_Wh"])
    battv_full = _f32(p["vatt_bf"]) + _f32(p["vatt_bh"])  # [AV]
    vatt_w = _f32(p["vatt_w"])[0]  # [AV]
    satt_Wx = _f32(p["satt_Wx"]); satt_Wh = _f32(p["satt_Wh"])
    bsatt_full = _f32(p["satt_bx"]) + _f32(p["satt_bh"])
    satt_w = _f32(p["satt_w"])[0]
    satt_b = float(_f32(p["satt_b"])[0])

    imgflat = img_s.reshape(B * R, F)  # (b,r)-major
    imgT = np.ascontiguousarray(imgflat.T)  # [F, B*R]

    in_maps = []
    for j in range(NC):
        rows = np.concatenate([np.arange(D * g + HS * j, D * g + HS * (j + 1))
                               for g in range(4)])  # 512 gate rows
        vrows = np.arange(VS * j, VS * (j + 1))

        def gate_slab(Wih_cols, Whh):
            # [K, GS] streams: rows = K-dim
            return np.concatenate([Wih_cols[rows].T, Whh[rows].T], axis=0)

        w1v_a = gate_slab(l1v_Wih[:, 0:1024], l1v_Whh)          # [2048, 512]
        w1s_a = gate_slab(l1s_Wih[:, 0:1024], l1s_Whh)
        w2_a = np.concatenate([l2_Wih[rows, 2048:3072].T,
                               l2_Wih[rows, 3584:4608].T,
                               l2_Whh[rows].T], axis=0)          # [3072, 512]

        # att1 stationary rows for this core (padded row tiles)
        att_rows = np.zeros((RPC * 128, F), np.float32)
        r0 = 128 * RPC * j
        r1 = min(r0 + 128 * RPC, B * R)
        if r1 > r0:
            att_rows[0:r1 - r0] = imgflat[r0:r1]
        # chunk-major stationary layout [128, 16*RPC*128]:
        # slice (c, rt) at cols (c*RPC+rt)*128
        ia = att_rows.reshape(RPC, 128, 16, 128)  # [rt, row, c, k]
        ia = ia.transpose(3, 2, 0, 1).reshape(128, 16 * RPC * 128)

        emb_cols_v = l1v_Wih[rows, 3072:4096]  # [512, 1024]
        emb_cols_s = l1s_Wih[rows, 1536:2560]

        wvt = np.stack([vatt_w[0:256], vatt_w[256:512]])  # [2, 256]
        wvt = np.repeat(wvt, B, axis=0)  # [128, 256]
        b2s = b2_full[rows]  # [512]
        b2t = np.stack([b2s[0:256], b2s[256:512]])
        b2t = np.repeat(b2t, B, axis=0)

        maskp = np.zeros((128, NPAIR), np.float32)
        for pp_ in range(NPAIR):
            maskp[0:B, pp_] = mask[:, 2 * pp_]
            if 2 * pp_ + 1 < T:
                maskp[B:128, pp_] = mask[:, 2 * pp_ + 1]

        in_maps.append({
            "w1v": _bf(_chunked(w1v_a)),
            "w1s": _bf(_chunked(w1s_a)),
            "w2": _bf(_chunked(w2_a)),
            "whv": _bf(_chunked(np.ascontiguousarray(vatt_Wh.T))),
            "whs": _bf(_chunked(np.ascontiguousarray(satt_Wh.T))),
            "wvv": _bf(_chunked(np.ascontiguousarray(wnv[vrows].T))),
            "wvs_d": _bf(_chunked(np.ascontiguousarray(wns[vrows].T))),
            "wvp_d": _bf(_chunked(np.ascontiguousarray(wnp[vrows].T))),
            "imgt": _bf(imgT),
            "imgt_att": _bf(ia),
            "embt": _bf(_chunked(np.ascontiguousarray(emb_tb.T))),
            "wembv": _bf(_chunked(np.ascontiguousarray(emb_cols_v.T))),
            "wembs": _bf(_chunked(np.ascontiguousarray(emb_cols_s.T))),
            "wimg": _bf(_chunked(
                np.ascontiguousarray(l1v_Wih[rows, 1024:3072].T) / 36.0)),
            "wzv": _bf(_chunked(np.ascontiguousarray(l2_Wih[rows, 0:2048].T))),
            "wf": _bf(_chunked(np.ascontiguousarray(vatt_Wf.T))),
            "amt": _bf(_chunked(np.ascontiguousarray(attr_mean.T))),
            "wattr1s": _bf(_chunked(
                np.ascontiguousarray(l1s_Wih[rows, 1024:1536].T))),
            "wattr2": _bf(_chunked(
                np.ascontiguousarray(l2_Wih[rows, 3072:3584].T))),
            "wsattx": _bf(_chunked(np.ascontiguousarray(satt_Wx.T))),
            "b1v": _f32(np.tile(b1v_full[rows][None, :], (B, 1))),
            "b1s": _f32(np.tile(b1s_full[rows][None, :], (B, 1))),
            "battv": _f32(np.tile(battv_full[None, :], (128, 1))),
            "bsatt": _f32(np.tile(bsatt_full[None, :], (B, 1))),
            "b2t": _f32(b2t),
            "wvtile": _bf(wvt),
            "wstile": _f32(np.tile(satt_w[None, :], (B, 1))),
            "sbt": np.full((B, 1), satt_b, np.float32),
            "maskp": maskp,
            "foldm": _bf(np.tile(np.eye(B, dtype=np.float32), (2, 1))),
        })
    # vocab biases applied host-side after gather (cheap, avoids device work)
    biases = (bp, bv, bs)
    return in_maps, caps_s, dec_len, sort_ind, mask, biases


_NC_CACHE = {}


def kernel(image_features, encoded_captions, caption_lengths,
           encoded_attributes, params):
    in_maps, caps_s, dec_len, sort_ind, mask, biases = prep_inputs(
        image_features, encoded_captions, caption_lengths,
        encoded_attributes, params)
    if "nc" not in _NC_CACHE:
        _NC_CACHE["nc"] = build_program()
    nc = _NC_CACHE["nc"]
    res = run_bass_kernel_spmd(nc, in_maps, core_ids=list(range(NC)))
    bp, bv, bs = biases
    mask3 = mask.T[:, :, None]  # [T, B, 1]

    def gather(name, bias):
        full = np.concatenate(
            [res.results[j][name].reshape(T, B, VS) for j in range(NC)],
            axis=2)  # [T, B, VCAP]
        full = full + bias[None, None, :] * mask3
        return np.ascontiguousarray(full.transpose(1, 0, 2))

    P = gather("pp_out", bp)
    Pv = gather("pv_out", bv)
    Ps = gather("ps_out", bs)
    caps_out = caps_s.astype(np.int32)
    dec_out = np.asarray(dec_len).astype(np.int32)
    si_out = np.asarray(sort_ind).astype(np.int32)
    return (P, Pv, Ps, caps_out, dec_out, si_out)
